# revision 11
# baseline (speedup 1.0000x reference)
"""Two-layer GCN (DGL GraphConv norm='both') on 8 Trainium2 NeuronCores.

Strategy (graph/data parallel, per sharding hint):
  - Nodes are range-partitioned across the 8 cores (1250 each); each core
    owns the dst-side segment_sum for its node range.
  - Host pre-sorts each core's incoming edges by dst, groups them into
    128-row dst windows, and pads each window's edge list to a uniform
    block count so all 8 cores share one SPMD program.
  - Layer-1 aggregation: dma_gather pulls (norm_src-scaled) source feature
    rows (bf16) from HBM; a per-block one-hot matrix M (built on-chip with
    iota + is_equal from precomputed local-dst ids) turns the segment sum
    into TensorEngine matmuls accumulating in PSUM:
        agg[dst,:] += M[edge,dst]^T @ Xg[edge,:]
  - H1 = (agg @ W1) * norm_dst + b1 (norm commutes through W1), ReLU, then
    z = (relu * norm_src) @ W2 is computed locally and AllGathered (bf16,
    padded to 128 cols so gather rows are 256B-aligned).
  - Layer-2 aggregation: same gather + one-hot matmul scheme over z,
    then out = agg2 * norm_dst + b2.
"""

import numpy as np
import ml_dtypes

BF16 = ml_dtypes.bfloat16
N_CORES = 8

LAST_STATS = {}


def _prep(features, W1, b1, W2, b2, src, dst):
    N, F = features.shape
    H = W1.shape[1]
    C = W2.shape[1]
    E = src.shape[0]
    assert N % N_CORES == 0
    npc = N // N_CORES            # nodes per core
    NT = (npc + 127) // 128       # dst windows per core
    npc_pad = NT * 128

    src = np.asarray(src, np.int64)
    dst = np.asarray(dst, np.int64)

    out_deg = np.bincount(src, minlength=N).astype(np.float32)
    in_deg = np.bincount(dst, minlength=N).astype(np.float32)
    norm_src = (1.0 / np.sqrt(np.clip(out_deg, 1.0, None))).astype(np.float32)
    norm_dst = (1.0 / np.sqrt(np.clip(in_deg, 1.0, None))).astype(np.float32)

    # norm_src folded into the gathered feature table (host-side sharding prep)
    featb = (np.asarray(features, np.float32) * norm_src[:, None]).astype(BF16)

    order = np.argsort(dst, kind="stable")
    ds = dst[order]
    ss = src[order]

    # per (core, window) counts
    i0 = np.empty((N_CORES, NT), np.int64)
    i1 = np.empty((N_CORES, NT), np.int64)
    for c in range(N_CORES):
        for w in range(NT):
            lo = c * npc + w * 128
            hi = min(lo + 128, (c + 1) * npc)
            i0[c, w] = np.searchsorted(ds, lo, "left")
            i1[c, w] = np.searchsorted(ds, hi, "left")
    counts = i1 - i0
    B = int(np.ceil(counts.max() / 128))      # blocks per window (uniform)
    EW = B * 128                              # padded edges per window
    NE = NT * EW                              # padded edges per core
    NBLK = NT * B

    idx1 = np.zeros((N_CORES, 128, NE // 16), np.int16)
    idx2 = np.zeros((N_CORES, 128, NE // 16), np.int16)
    dlw = np.zeros((N_CORES, 128, NBLK), BF16)
    ndst_t = np.zeros((N_CORES, 128, NT), np.float32)
    nso_t = np.zeros((N_CORES, 128, NT), np.float32)

    for c in range(N_CORES):
        s1 = np.zeros(NE, np.int64)
        dl = np.full(NE, -1.0, np.float32)
        for w in range(NT):
            a, b = i0[c, w], i1[c, w]
            cnt = b - a
            pos = w * EW
            s1[pos:pos + cnt] = ss[a:b]
            dl[pos:pos + cnt] = (ds[a:b] - (c * npc + w * 128)).astype(np.float32)
        s2 = (s1 // npc) * npc_pad + (s1 % npc)   # row in AllGathered z
        idx1[c] = np.tile(s1.reshape(NE // 16, 16).T.astype(np.int16), (8, 1))
        idx2[c] = np.tile(s2.reshape(NE // 16, 16).T.astype(np.int16), (8, 1))
        dlw[c] = dl.reshape(NBLK, 128).T.astype(BF16)

        own_nd = np.ones(npc_pad, np.float32)
        own_nd[:npc] = norm_dst[c * npc:(c + 1) * npc]
        own_ns = np.ones(npc_pad, np.float32)
        own_ns[:npc] = norm_src[c * npc:(c + 1) * npc]
        ndst_t[c] = own_nd.reshape(NT, 128).T
        nso_t[c] = own_ns.reshape(NT, 128).T

    shapes = dict(N=N, F=F, H=H, C=C, NT=NT, B=B, NE=NE, npc=npc)
    in_maps = []
    W1b = np.asarray(W1, np.float32).astype(BF16)
    W2b = np.asarray(W2, np.float32).astype(BF16)
    b1f = np.asarray(b1, np.float32)[None, :]
    b2f = np.asarray(b2, np.float32)[None, :]
    for c in range(N_CORES):
        in_maps.append(dict(
            featb=featb, idx1=idx1[c], idx2=idx2[c], dlbf=dlw[c],
            ndst=ndst_t[c], nso=nso_t[c],
            w1=W1b, w2=W2b, b1=b1f, b2=b2f,
        ))
    return shapes, in_maps


def _build(N, F, H, C, NT, B, NE, variant="full", num_devices=N_CORES,
           repeat=1):
    import concourse.bacc as bacc
    import concourse.mybir as mybir
    from concourse import tile

    dt = mybir.dt
    alu = mybir.AluOpType
    KF = F // 128   # feat chunks
    KH = H // 128   # hidden chunks
    EW = B * 128
    ZROWS = N_CORES * NT * 128

    nc = bacc.Bacc("TRN2", target_bir_lowering=False, debug=False,
                   num_devices=num_devices, num_swdge_queues=4)

    featb_d = nc.dram_tensor("featb", [N, F], dt.bfloat16, kind="ExternalInput")
    idx1_d = nc.dram_tensor("idx1", [128, NE // 16], dt.int16, kind="ExternalInput")
    idx2_d = nc.dram_tensor("idx2", [128, NE // 16], dt.int16, kind="ExternalInput")
    dlbf_d = nc.dram_tensor("dlbf", [128, NT * B], dt.bfloat16, kind="ExternalInput")
    ndst_d = nc.dram_tensor("ndst", [128, NT], dt.float32, kind="ExternalInput")
    nso_d = nc.dram_tensor("nso", [128, NT], dt.float32, kind="ExternalInput")
    w1_d = nc.dram_tensor("w1", [F, H], dt.bfloat16, kind="ExternalInput")
    w2_d = nc.dram_tensor("w2", [H, C], dt.bfloat16, kind="ExternalInput")
    b1_d = nc.dram_tensor("b1", [1, H], dt.float32, kind="ExternalInput")
    b2_d = nc.dram_tensor("b2", [1, C], dt.float32, kind="ExternalInput")
    out_d = nc.dram_tensor("out", [NT * 128, C], dt.float32, kind="ExternalOutput")

    with tile.TileContext(nc) as tc:
        with (
            tc.tile_pool(name="const", bufs=1) as const,
            tc.tile_pool(name="dram", bufs=1, space="DRAM") as dram,
            tc.tile_pool(name="xg", bufs=3) as xg_pool,
            tc.tile_pool(name="mp", bufs=3) as m_pool,
            tc.tile_pool(name="work", bufs=3) as work,
            tc.tile_pool(name="ps_agg", bufs=2, space="PSUM") as ps_agg,
            tc.tile_pool(name="ps_tr", bufs=2, space="PSUM") as ps_tr,
            tc.tile_pool(name="ps_h", bufs=2, space="PSUM") as ps_h,
        ):
            # ---- constants ----
            idx1_t = const.tile([128, NE // 16], dt.int16)
            nc.sync.dma_start(idx1_t[:], idx1_d.ap())
            idx2_t = const.tile([128, NE // 16], dt.int16)
            nc.sync.dma_start(idx2_t[:], idx2_d.ap())
            dlbf_t = const.tile([128, NT * B], dt.bfloat16)
            nc.sync.dma_start(dlbf_t[:], dlbf_d.ap())
            ndst_t = const.tile([128, NT], dt.float32)
            nc.sync.dma_start(ndst_t[:], ndst_d.ap())
            nso_t = const.tile([128, NT], dt.float32)
            nc.sync.dma_start(nso_t[:], nso_d.ap())

            w1_sb = const.tile([128, KF, H], dt.bfloat16)
            nc.sync.dma_start(w1_sb[:], w1_d.ap().rearrange("(k p) n -> p k n", p=128))
            w2_sb = const.tile([128, KH, C], dt.bfloat16)
            nc.sync.dma_start(w2_sb[:], w2_d.ap().rearrange("(k p) n -> p k n", p=128))
            b1_sb = const.tile([1, H], dt.float32)
            nc.sync.dma_start(b1_sb[:], b1_d.ap())
            b2_sb = const.tile([1, C], dt.float32)
            nc.sync.dma_start(b2_sb[:], b2_d.ap())

            iota_bf = const.tile([128, B, 128], dt.bfloat16)
            nc.gpsimd.iota(iota_bf[:], pattern=[[0, B], [1, 128]], base=0,
                           channel_multiplier=0,
                           allow_small_or_imprecise_dtypes=True)
            iota_col = const.tile([128, 1], dt.float32)
            nc.gpsimd.iota(iota_col[:], pattern=[[0, 1]], base=0,
                           channel_multiplier=1,
                           allow_small_or_imprecise_dtypes=True)
            ident_bf = const.tile([128, 128], dt.bfloat16)
            nc.vector.tensor_scalar(ident_bf[:], iota_bf[:, 0, :], iota_col[:],
                                    None, alu.is_equal)

            # bias rows broadcast across partitions via ones-column matmul
            ones_sb = const.tile([1, 128], dt.float32)
            nc.vector.memset(ones_sb[:], 1.0)
            b1_ps = ps_h.tile([128, H], dt.float32, tag="h")
            nc.tensor.matmul(b1_ps[:], lhsT=ones_sb[:], rhs=b1_sb[:],
                             start=True, stop=True)
            b1_bc = const.tile([128, H], dt.float32)
            nc.vector.tensor_copy(b1_bc[:], b1_ps[:])
            b2_ps = ps_h.tile([128, C], dt.float32, tag="zn")
            nc.tensor.matmul(b2_ps[:], lhsT=ones_sb[:], rhs=b2_sb[:],
                             start=True, stop=True)
            b2_bc = const.tile([128, C], dt.float32)
            nc.vector.tensor_copy(b2_bc[:], b2_ps[:])

            # z rows padded to 256 bf16 cols = 512B: dma_gather rows below
            # 512B crash the Q7 ucode; gathered cols C.. are never consumed.
            cc_in = dram.tile([NT * 128, 256], dt.bfloat16)
            z_fulls = [dram.tile([ZROWS, 256], dt.bfloat16, addr_space="Shared",
                                 name=f"z_full_r{_r}")
                       for _r in range(repeat)]
            z_full = z_fulls[0]

            # ---- layer 1 ----
            # dma_gather is limited to 1024 idxs per instruction (64 descs per
            # SDMA lane = the single_packet ceiling); larger crashes the Q7.
            GC = 8  # blocks (of 128 idxs) per gather instruction

            def _l1(w):
                xg = xg_pool.tile([128, B, F], dt.bfloat16, tag="xg")
                if variant == "no_gather":
                    nc.vector.memset(xg[:], 0.25)
                elif variant == "no_l1gather":
                    nc.vector.memset(xg[:, 0, 0:16], 0.25)
                else:
                    for g in range(0, B, GC):
                        nb = min(GC, B - g)
                        c0 = (w * B + g) * 8
                        nc.gpsimd.dma_gather(
                            xg[:, g:g + nb, :], featb_d.ap(),
                            idx1_t[:, c0:c0 + nb * 8],
                            nb * 128, nb * 128, F,
                            queue_num=(w * ((B + GC - 1) // GC) + g // GC) % 4)
                m1 = m_pool.tile([128, B, 128], dt.bfloat16, tag="m")
                nc.vector.tensor_tensor(
                    m1[:], iota_bf[:],
                    dlbf_t[:, w * B:(w + 1) * B].broadcast_to((128, B, 128)),
                    alu.is_equal)
                agg = ps_agg.tile([128, F], dt.float32, tag="agg")
                NMM = 32 if variant == "tiny_mm" else F
                for b in range(B):
                    nc.tensor.matmul(agg[:, 0:NMM], lhsT=m1[:, b, :],
                                     rhs=xg[:, b, 0:NMM],
                                     start=(b == 0), stop=(b == B - 1))
                aggc = work.tile([128, F], dt.bfloat16, tag="aggc")
                nc.vector.tensor_copy(aggc[:], agg[:])
                if variant == "no_tail":
                    nc.sync.dma_start(cc_in[w * 128:(w + 1) * 128, :], aggc[:])
                    return
                aggT = work.tile([128, KF, 128], dt.bfloat16, tag="aggT")
                for k in range(KF):
                    trp = ps_tr.tile([128, 128], dt.bfloat16, tag="tr")
                    nc.tensor.transpose(trp[:], aggc[:, k * 128:(k + 1) * 128],
                                        ident_bf[:])
                    nc.vector.tensor_copy(aggT[:, k, :], trp[:])
                h1 = ps_h.tile([128, H], dt.float32, tag="h")
                for k in range(KF):
                    nc.tensor.matmul(h1[:], lhsT=aggT[:, k, :], rhs=w1_sb[:, k, :],
                                     start=(k == 0), stop=(k == KF - 1))
                t1 = work.tile([128, H], dt.float32, tag="t1")
                nc.vector.scalar_tensor_tensor(t1[:], h1[:], ndst_t[:, w:w + 1],
                                               b1_bc[:], alu.mult, alu.add)
                yz = work.tile([128, H], dt.bfloat16, tag="yz")
                nc.scalar.activation(yz[:], t1[:],
                                     mybir.ActivationFunctionType.Relu,
                                     scale=nso_t[:, w:w + 1])
                yzT = work.tile([128, KH, 128], dt.bfloat16, tag="yzT")
                for k in range(KH):
                    trp2 = ps_tr.tile([128, 128], dt.bfloat16, tag="tr")
                    nc.tensor.transpose(trp2[:], yz[:, k * 128:(k + 1) * 128],
                                        ident_bf[:])
                    nc.vector.tensor_copy(yzT[:, k, :], trp2[:])
                zn = ps_h.tile([128, C], dt.float32, tag="zn")
                for k in range(KH):
                    nc.tensor.matmul(zn[:], lhsT=yzT[:, k, :], rhs=w2_sb[:, k, :],
                                     start=(k == 0), stop=(k == KH - 1))
                znb = work.tile([128, 256], dt.bfloat16, tag="znb")
                nc.vector.memset(znb[:], 0.0)
                nc.vector.tensor_copy(znb[:, :C], zn[:])
                nc.sync.dma_start(cc_in[w * 128:(w + 1) * 128, :], znb[:])

            # ---- halo exchange ----
            def _halo(z_full):
                if variant == "no_cc" or num_devices == 1:
                    nc.sync.dma_start(z_full[0:NT * 128, :], cc_in[:, :])
                else:
                    nc.gpsimd.collective_compute(
                        "AllGather", alu.bypass,
                        replica_groups=[list(range(N_CORES))],
                        ins=[cc_in.opt()], outs=[z_full.opt()])

            # ---- layer 2 ----
            def _l2(w, z_full):
                # z rows sit at 512B pitch in z_full; read only the first
                # 256B (the C real cols + pad) of each — elem_step > elem_size.
                xg2 = xg_pool.tile([128, B, 128], dt.bfloat16, tag="xg2")
                if variant == "no_l2gather_pure":
                    nc.vector.memset(xg2[:, 0, 0:16], 0.25)
                elif variant in ("no_gather", "no_l2gather"):
                    nc.vector.memset(xg2[:], 0.25)
                else:
                    for g in range(0, B, GC):
                        nb = min(GC, B - g)
                        c0 = (w * B + g) * 8
                        nc.gpsimd.dma_gather(
                            xg2[:, g:g + nb, :], z_full[:, 0:128],
                            idx2_t[:, c0:c0 + nb * 8],
                            nb * 128, nb * 128, 128, elem_step=256,
                            queue_num=(w * ((B + GC - 1) // GC) + g // GC) % 4)
                m2 = m_pool.tile([128, B, 128], dt.bfloat16, tag="m")
                nc.vector.tensor_tensor(
                    m2[:], iota_bf[:],
                    dlbf_t[:, w * B:(w + 1) * B].broadcast_to((128, B, 128)),
                    alu.is_equal)
                agg2 = ps_agg.tile([128, C], dt.float32, tag="agg")
                for b in range(B):
                    nc.tensor.matmul(agg2[:], lhsT=m2[:, b, :],
                                     rhs=xg2[:, b, 0:C],
                                     start=(b == 0), stop=(b == B - 1))
                ot = work.tile([128, C], dt.float32, tag="ot")
                nc.vector.scalar_tensor_tensor(ot[:], agg2[:],
                                               ndst_t[:, w:w + 1], b2_bc[:],
                                               alu.mult, alu.add)
                nc.sync.dma_start(out_d.ap()[w * 128:(w + 1) * 128, :], ot[:])

            for _rep in range(repeat):
                for w in range(NT):
                    _l1(w)
                _halo(z_fulls[_rep])
                for w in range(NT):
                    _l2(w, z_fulls[_rep])

    nc.compile()
    return nc


def kernel(features, W1, b1, W2, b2, src, dst, **_):
    import time
    from concourse.bass_utils import run_bass_kernel_spmd

    t0 = time.time()
    shapes, in_maps = _prep(features, W1, b1, W2, b2, src, dst)
    t1 = time.time()
    nc = _build(shapes["N"], shapes["F"], shapes["H"], shapes["C"],
                shapes["NT"], shapes["B"], shapes["NE"])
    t2 = time.time()
    res = run_bass_kernel_spmd(nc, in_maps, core_ids=list(range(N_CORES)))
    t3 = time.time()
    npc = shapes["npc"]
    out = np.concatenate([res.results[c]["out"][:npc] for c in range(N_CORES)], 0)
    LAST_STATS.update(prep_s=t1 - t0, build_s=t2 - t1, run_s=t3 - t2,
                      B=shapes["B"], NE=shapes["NE"])
    return np.ascontiguousarray(out.astype(np.float32))



# revision 12
# speedup vs baseline: 12.0375x; 12.0375x over previous
"""Two-layer GCN (DGL GraphConv norm='both') on 8 Trainium2 NeuronCores.

Dense-adjacency formulation (replaces the per-edge dma_gather kernel, which
was descriptor-bound at ~70 GB/s):
  - Nodes range-partitioned across 8 cores (1250 each, padded to 1280).
  - Each core holds its column-slice of the global adjacency as a DENSE
    fp8 matrix A_T[src_pad=10240, dst=1280] (entries = edge multiplicity,
    exactly representable in fp8e4); norm_src is folded into the feature
    table, norm_dst applied after aggregation. A_T lives resident in SBUF
    (102 KB/partition) and serves BOTH layers.
  - Layer 1 computes aggT[f, dst] = sum_j featb[:,j,f]^T @ A_T[:,j,dst]
    directly in transposed layout, so the W1 GEMM, relu and W2 GEMM need
    no on-chip transposes; bias terms enter as rank-1 (K=1) matmuls that
    initialize the PSUM accumulators.
  - z (layer-1 output, bf16) is AllGathered (10240 x 64), re-loaded as a
    single contiguous DMA, and layer 2 aggregates with the same A_T.
  - Src index mapping: global padded row r sits at SBUF partition r//80,
    block r%80 (the layout a flat [10240, .] row-major DMA produces).
"""

import numpy as np
import ml_dtypes

BF16 = ml_dtypes.bfloat16
N_CORES = 8

LAST_STATS = {}


def _prep(features, W1, b1, W2, b2, src, dst):
    N, F = features.shape
    H = W1.shape[1]
    C = W2.shape[1]
    assert N % N_CORES == 0
    npc = N // N_CORES              # nodes per core (1250)
    NT = (npc + 127) // 128         # dst windows per core (10)
    npp = NT * 128                  # padded nodes per core (1280)
    NP = N_CORES * npp              # global padded (10240)
    JB = NP // 128                  # src blocks (80)
    DW = npp                        # dst width per core

    src = np.asarray(src, np.int64)
    dst = np.asarray(dst, np.int64)

    out_deg = np.bincount(src, minlength=N).astype(np.float32)
    in_deg = np.bincount(dst, minlength=N).astype(np.float32)
    norm_src = (1.0 / np.sqrt(np.clip(out_deg, 1.0, None))).astype(np.float32)
    norm_dst = (1.0 / np.sqrt(np.clip(in_deg, 1.0, None))).astype(np.float32)

    # feature table in padded global layout, norm_src folded in
    featp = np.zeros((NP, F), BF16)
    fsc = (np.asarray(features, np.float32) * norm_src[:, None]).astype(BF16)
    for c in range(N_CORES):
        featp[c * npp:c * npp + npc] = fsc[c * npc:(c + 1) * npc]

    # padded global src index of each edge
    sp = (src // npc) * npp + (src % npc)
    core_of_dst = dst // npc
    dl = dst - core_of_dst * npc        # local dst 0..npc-1

    FP8 = np.dtype(ml_dtypes.float8_e4m3)
    lut = np.arange(256).astype(np.float32).astype(FP8)

    in_maps = []
    w1b = np.asarray(W1, np.float32).astype(BF16)
    w2b = np.asarray(W2, np.float32).astype(BF16)
    b1r = np.asarray(b1, np.float32)[None, :]
    b2r = np.asarray(b2, np.float32)[None, :]
    for c in range(N_CORES):
        m = core_of_dst == c
        flat = sp[m] * DW + dl[m]
        counts = np.bincount(flat, minlength=NP * DW)
        mx = counts.max()
        assert mx < 16, f"edge multiplicity {mx} not fp8-exact"
        at = lut[counts.astype(np.uint8)].reshape(128, JB * DW)

        ndl = np.zeros(DW, np.float32)
        ndl[:npc] = norm_dst[c * npc:(c + 1) * npc]
        nsl = np.zeros(DW, np.float32)
        nsl[:npc] = norm_src[c * npc:(c + 1) * npc]
        nnr = (ndl * nsl)[None, :]                      # relu scale (layer 1)
        indr = np.zeros(DW, np.float32)
        indr[:npc] = 1.0 / ndl[:npc]                    # bias bake 1/norm_dst
        ndt = ndl.reshape(NT, 128).T.copy()             # [128, NT] column form

        in_maps.append(dict(
            featp=featp, at=at, w1=w1b, w2=w2b, b1r=b1r, b2r=b2r,
            nnr=nnr, indr=indr[None, :], ndt=ndt, ndr=ndl[None, :],
        ))

    shapes = dict(N=N, F=F, H=H, C=C, NT=NT, npc=npc, NP=NP, JB=JB, DW=DW)
    return shapes, in_maps


def _build(N, F, H, C, NT, NP, JB, DW, variant="full", num_devices=N_CORES,
           repeat=1):
    import concourse.bacc as bacc
    import concourse.mybir as mybir
    from concourse import tile

    dt = mybir.dt
    alu = mybir.AluOpType
    FC = F // 128
    HC = H // 128
    ACHUNK = 10                     # src blocks per A_T load chunk
    FCHUNK = 20                     # src blocks per feature load chunk
    DCS = [(d, min(d + 512, DW)) for d in range(0, DW, 512)]

    nc = bacc.Bacc("TRN2", target_bir_lowering=False, debug=False,
                   num_devices=num_devices, num_swdge_queues=4)

    featp_d = nc.dram_tensor("featp", [NP, F], dt.bfloat16, kind="ExternalInput")
    at_d = nc.dram_tensor("at", [128, JB * DW], dt.float8e4, kind="ExternalInput")
    w1_d = nc.dram_tensor("w1", [F, H], dt.bfloat16, kind="ExternalInput")
    w2_d = nc.dram_tensor("w2", [H, C], dt.bfloat16, kind="ExternalInput")
    b1_d = nc.dram_tensor("b1r", [1, H], dt.float32, kind="ExternalInput")
    b2_d = nc.dram_tensor("b2r", [1, C], dt.float32, kind="ExternalInput")
    nn_d = nc.dram_tensor("nnr", [1, DW], dt.float32, kind="ExternalInput")
    ind_d = nc.dram_tensor("indr", [1, DW], dt.float32, kind="ExternalInput")
    ndt_d = nc.dram_tensor("ndt", [128, NT], dt.float32, kind="ExternalInput")
    ndr_d = nc.dram_tensor("ndr", [1, DW], dt.float32, kind="ExternalInput")
    out_d = nc.dram_tensor("out", [NT * 128, C], dt.float32, kind="ExternalOutput")

    with tile.TileContext(nc) as tc:
        with (
            tc.tile_pool(name="const", bufs=1) as const,
            tc.tile_pool(name="dram", bufs=1, space="DRAM") as dram,
            tc.tile_pool(name="work", bufs=3) as work,
            tc.tile_pool(name="ps1", bufs=1, space="PSUM") as ps1,
            tc.tile_pool(name="ps2", bufs=2, space="PSUM") as ps2,
        ):
            # ---- constants ----
            w1_sb = const.tile([128, FC, H], dt.bfloat16)
            nc.sync.dma_start(w1_sb[:], w1_d.ap().rearrange("(k p) n -> p k n", p=128))
            w2_sb = const.tile([128, HC, C], dt.bfloat16)
            nc.sync.dma_start(w2_sb[:], w2_d.ap().rearrange("(k p) n -> p k n", p=128))
            b1r = const.tile([1, H], dt.float32)
            nc.sync.dma_start(b1r[:], b1_d.ap())
            b2r = const.tile([1, C], dt.float32)
            nc.sync.dma_start(b2r[:], b2_d.ap())
            nnr = const.tile([1, DW], dt.float32)
            nc.sync.dma_start(nnr[:], nn_d.ap())
            indr = const.tile([1, DW], dt.float32)
            nc.sync.dma_start(indr[:], ind_d.ap())
            ndt = const.tile([128, NT], dt.float32)
            nc.sync.dma_start(ndt[:], ndt_d.ap())
            ndr = const.tile([1, DW], dt.float32)
            nc.sync.dma_start(ndr[:], ndr_d.ap())
            ones = const.tile([1, 128], dt.float32)
            nc.vector.memset(ones[:], 1.0)

            if variant == "l2t":
                iota_bf = const.tile([128, 128], dt.bfloat16)
                nc.gpsimd.iota(iota_bf[:], pattern=[[1, 128]], base=0,
                               channel_multiplier=0,
                               allow_small_or_imprecise_dtypes=True)
                iota_col = const.tile([128, 1], dt.float32)
                nc.gpsimd.iota(iota_col[:], pattern=[[0, 1]], base=0,
                               channel_multiplier=1,
                               allow_small_or_imprecise_dtypes=True)
                ident_bf = const.tile([128, 128], dt.bfloat16)
                nc.vector.tensor_scalar(ident_bf[:], iota_bf[:], iota_col[:],
                                        None, alu.is_equal)

            # A_T resident in SBUF, chunked loads so compute can chase
            at_sb = []
            for k in range(JB // ACHUNK):
                t = const.tile([128, ACHUNK, DW], dt.float8e4, name=f"at_sb{k}")
                nc.sync.dma_start(
                    t[:], at_d.ap()[:, k * ACHUNK * DW:(k + 1) * ACHUNK * DW])
                at_sb.append(t)

            fb_sb = []
            fview = featp_d.ap().rearrange("(p j) f -> p j f", p=128)
            for k in range(JB // FCHUNK):
                t = const.tile([128, FCHUNK, F], dt.bfloat16, name=f"fb_sb{k}")
                nc.sync.dma_start(t[:], fview[:, k * FCHUNK:(k + 1) * FCHUNK, :])
                fb_sb.append(t)

            def at_blk(j, d0, d1):
                return at_sb[j // ACHUNK][:, j % ACHUNK, d0:d1]

            def fb_blk(j, lo, hi):
                return fb_sb[j // FCHUNK][:, j % FCHUNK, lo:hi]

            # nn broadcast tile [128, DW] via rank-1 matmuls
            nn_bc = const.tile([128, DW], dt.float32)
            for d0, d1 in DCS:
                bc_ps = ps1.tile([128, 512], dt.float32, tag="h10")
                nc.tensor.matmul(bc_ps[:, :d1 - d0], lhsT=ones[:],
                                 rhs=nnr[:, d0:d1], start=True, stop=True)
                nc.vector.tensor_copy(nn_bc[:, d0:d1], bc_ps[:, :d1 - d0])
            if variant == "l2t":
                nd_bc = const.tile([64, DW], dt.float32)
                for d0, d1 in DCS:
                    bc2 = ps1.tile([128, 512], dt.float32, tag="h10")
                    nc.tensor.matmul(bc2[0:64, :d1 - d0], lhsT=ones[:, 0:64],
                                     rhs=ndr[:, d0:d1], start=True, stop=True)
                    nc.vector.tensor_copy(nd_bc[:, d0:d1], bc2[0:64, :d1 - d0])

            cc_in = dram.tile([NT * 128, C], dt.bfloat16)
            z_fulls = [dram.tile([NP, C], dt.bfloat16, addr_space="Shared",
                                 name=f"z_full_r{_r}")
                       for _r in range(repeat)]

            yT_sb = const.tile([128, HC, DW], dt.bfloat16)
            ot_all = const.tile([128, NT, C], dt.float32)

            # ---- layer 1: one pass per 512-wide dst chunk ----
            def _l1(dci):
                d0, d1 = DCS[dci]
                dn = d1 - d0
                aggs = []
                for fc in range(FC):
                    agg = ps1.tile([128, 512], dt.float32, tag=f"agg{fc}")
                    if variant == "no_l1mm":
                        nc.tensor.matmul(agg[:, :dn], lhsT=fb_blk(0, fc * 128, fc * 128 + 128),
                                         rhs=at_blk(0, d0, d1), start=True, stop=True)
                    else:
                        for j in range(JB):
                            nc.tensor.matmul(agg[:, :dn],
                                             lhsT=fb_blk(j, fc * 128, fc * 128 + 128),
                                             rhs=at_blk(j, d0, d1),
                                             start=(j == 0), stop=(j == JB - 1))
                    asb = work.tile([128, 512], dt.bfloat16, tag=f"aggs{fc}")
                    nc.vector.tensor_copy(asb[:, :dn], agg[:, :dn])
                    aggs.append(asb)
                for hc in range(HC):
                    h1 = ps1.tile([128, 512], dt.float32, tag=f"h1{hc}")
                    nc.tensor.matmul(h1[:, :dn], lhsT=b1r[:, hc * 128:hc * 128 + 128],
                                     rhs=indr[:, d0:d1], start=True, stop=False)
                    for fc in range(FC):
                        nc.tensor.matmul(h1[:, :dn],
                                         lhsT=w1_sb[:, fc, hc * 128:hc * 128 + 128],
                                         rhs=aggs[fc][:, :dn],
                                         start=False, stop=(fc == FC - 1))
                    rel = work.tile([128, 512], dt.bfloat16, tag="rel")
                    nc.scalar.activation(rel[:, :dn], h1[:, :dn],
                                         mybir.ActivationFunctionType.Relu)
                    nc.vector.tensor_tensor(yT_sb[:, hc, d0:d1], rel[:, :dn],
                                            nn_bc[:, d0:d1], alu.mult)
                for w in range(d0 // 128, d1 // 128):
                    zp = ps2.tile([128, C], dt.float32, tag="z")
                    for hc in range(HC):
                        nc.tensor.matmul(zp[:], lhsT=yT_sb[:, hc, w * 128:(w + 1) * 128],
                                         rhs=w2_sb[:, hc, :],
                                         start=(hc == 0), stop=(hc == HC - 1))
                    znb = work.tile([128, C], dt.bfloat16, tag="znb")
                    nc.vector.tensor_copy(znb[:], zp[:])
                    nc.sync.dma_start(cc_in[w * 128:(w + 1) * 128, :], znb[:])

            # ---- halo exchange ----
            def _halo(z_full):
                if variant == "no_cc" or num_devices == 1:
                    nc.sync.dma_start(z_full[0:NT * 128, :], cc_in[:, :])
                else:
                    nc.gpsimd.collective_compute(
                        "AllGather", alu.bypass,
                        replica_groups=[list(range(N_CORES))],
                        ins=[cc_in.opt()], outs=[z_full.opt()])

            # ---- layer 2 ----
            def _l2(z_full, rep):
                z_sb = work.tile([128, JB, C], dt.bfloat16, tag="z_sb", bufs=2)
                nc.sync.dma_start(z_sb[:],
                                  z_full.rearrange("(p j) c -> p j c", p=128))
                for w in range(NT):
                    agg2 = ps2.tile([128, C], dt.float32, tag="agg2")
                    nc.tensor.matmul(agg2[:], lhsT=indr[:, w * 128:(w + 1) * 128],
                                     rhs=b2r[:], start=True, stop=False)
                    if variant == "no_l2mm":
                        nc.tensor.matmul(agg2[:], lhsT=at_blk(0, w * 128, (w + 1) * 128),
                                         rhs=z_sb[:, 0, :], start=False, stop=True)
                    else:
                        for j in range(JB):
                            nc.tensor.matmul(agg2[:],
                                             lhsT=at_blk(j, w * 128, (w + 1) * 128),
                                             rhs=z_sb[:, j, :],
                                             start=False, stop=(j == JB - 1))
                    nc.vector.tensor_scalar(ot_all[:, w, :], agg2[:],
                                            ndt[:, w:w + 1], None, alu.mult)
                nc.sync.dma_start(
                    out_d.ap().rearrange("(w p) c -> p w c", p=128), ot_all[:])

            for _rep in range(repeat):
                for dci in range(len(DCS)):
                    _l1(dci)
                _halo(z_fulls[_rep])
                _l2(z_fulls[_rep], _rep)

    nc.compile()
    return nc


def kernel(features, W1, b1, W2, b2, src, dst, **_):
    import time
    from concourse.bass_utils import run_bass_kernel_spmd

    t0 = time.time()
    shapes, in_maps = _prep(features, W1, b1, W2, b2, src, dst)
    t1 = time.time()
    nc = _build(shapes["N"], shapes["F"], shapes["H"], shapes["C"],
                shapes["NT"], shapes["NP"], shapes["JB"], shapes["DW"])
    t2 = time.time()
    res = run_bass_kernel_spmd(nc, in_maps, core_ids=list(range(N_CORES)))
    t3 = time.time()
    npc = shapes["npc"]
    out = np.concatenate([res.results[c]["out"][:npc] for c in range(N_CORES)], 0)
    LAST_STATS.update(prep_s=t1 - t0, build_s=t2 - t1, run_s=t3 - t2)
    return np.ascontiguousarray(out.astype(np.float32))
